# revision 1
# baseline (speedup 1.0000x reference)
"""GCN (2-layer + linear head + log_softmax) on 8 Trainium2 NeuronCores.

Strategy (graph/data parallel, per sharding hint):
  - Nodes are partitioned across the 8 cores (degree-sorted serpentine for
    edge balance).  The small weights are replicated.
  - Per GCN layer:  h = x_shard @ W  on each core (PE), scale rows by
    dinv = (deg+1)^-1/2, AllGather the scaled rows ("halo exchange" of the
    full node-feature table, fp16) into a DRAM table, then each core
    aggregates its destination shard with indirect-DMA row gathers + a DVE
    pairwise tree-sum over a degree-padded (ELL) layout.
  - Math identity used:  out[d] = dinv[d] * sum_{e:dst=d} dinv[src]*h[src]
                                  + dinv[d]^2 * h[d] + b
    so no per-edge coefficients are needed, only per-node scaling.

Host-side numpy does only graph-structure preprocessing (degree counting,
node permutation, padded gather-index construction) and output unpermute.
All floating-point tensor math runs on the NeuronCores.
"""

import math
import os

import numpy as np

import concourse.bass as bass
import concourse.mybir as mybir
import concourse.tile as tile
from concourse.bass import IndirectOffsetOnAxis
from concourse.bass_utils import run_bass_kernel_spmd
from concourse.masks import make_identity

FP16 = mybir.dt.float16
F32 = mybir.dt.float32
I32 = mybir.dt.int32

N_CORES = 8
F_DIM = 64  # in = hidden = 64
C_DIM = 16

# slots per partition per gather chunk (bounds SBUF gather tile)
S_MAX = 96

# depth quantization ladder (round group depth up to one of these)
_LADDER = list(range(1, 17)) + [18, 20, 22, 24, 26, 28, 32, 36, 40, 48, 56, 64, 80, 96, 128]


def _quantize_depth(d):
    for v in _LADDER:
        if d <= v:
            return v
    return int(d)  # very high degree: exact (single-group chunks)


class _Plan:
    pass


def build_plan(edge_index, n_nodes):
    """Host-side graph preprocessing.  Pure index math, O(E)."""
    src = np.asarray(edge_index[0], dtype=np.int64)
    dst = np.asarray(edge_index[1], dtype=np.int64)
    E = src.shape[0]

    deg = np.bincount(dst, minlength=n_nodes).astype(np.int64)

    # Degree-descending global order; serpentine core assignment balances
    # both node count and edge count per core, and gives each core a
    # degree-sorted shard (tight ELL padding).
    order = np.argsort(-deg, kind="stable")
    pos = np.arange(n_nodes)
    blk, lane = pos // N_CORES, pos % N_CORES
    core_of_pos = np.where(blk % 2 == 0, lane, N_CORES - 1 - lane)
    nodes_per_core = [order[core_of_pos == c] for c in range(N_CORES)]
    n_shard_max = max(len(v) for v in nodes_per_core)

    SHR = ((n_shard_max + 127) // 128) * 128
    if SHR == n_shard_max:
        SHR += 128  # guarantee >=1 dummy (zero) row per shard for padding
    NG = SHR // 128

    # table row for node with in-core rank r (device position g=r//128,
    # p=r%128) is  c*SHR + p*NG + g  — partition-major, so the on-device
    # shard store [128, NG*64] -> [SHR, 64] is one contiguous run per
    # partition (2D DMA).
    new_id = np.full(n_nodes, -1, dtype=np.int64)
    for c, nl in enumerate(nodes_per_core):
        r = np.arange(len(nl))
        new_id[nl] = c * SHR + (r % 128) * NG + (r // 128)

    # shared (across cores) per-group depth
    depth = np.zeros(NG, dtype=np.int64)
    for nl in nodes_per_core:
        d = deg[nl]
        d = np.pad(d, (0, SHR - len(nl)))
        depth = np.maximum(depth, d.reshape(NG, 128).max(axis=1))
    depth = np.maximum(depth, 1)
    depth_q = np.array([_quantize_depth(int(d)) for d in depth], dtype=np.int64)

    # chunks: runs of equal quantized depth, split so G*D <= max(S_MAX, D)
    chunks = []  # (g0, g1, D, col_off)
    col = 0
    g = 0
    while g < NG:
        D = int(depth_q[g])
        run_end = g
        while run_end < NG and depth_q[run_end] == D:
            run_end += 1
        gpc = max(1, S_MAX // D)
        while g < run_end:
            g1 = min(run_end, g + gpc)
            chunks.append((g, g1, D, col))
            col += (g1 - g) * D
            g = g1
    S_total = col

    colbase = np.zeros(NG, dtype=np.int64)
    for (g0, g1, D, off) in chunks:
        colbase[g0:g1] = off + (np.arange(g1 - g0)) * D

    # per-edge slot assignment
    nd = new_id[dst]
    ns = new_id[src]
    o = np.argsort(nd, kind="stable")
    nd_s = nd[o]
    ns_s = ns[o]
    first = np.searchsorted(nd_s, nd_s, side="left")
    j = np.arange(E) - first
    c_e = nd_s // SHR
    row = nd_s % SHR
    p_e = row // NG
    g_e = row % NG
    col_e = colbase[g_e] + j
    assert (j < depth_q[g_e]).all()

    idx = np.empty((N_CORES, 128, S_total), dtype=np.int32)
    for c in range(N_CORES):
        idx[c, :, :] = c * SHR + (SHR - 1)  # pad -> own dummy (zero) row
    idx[c_e, p_e, col_e] = ns_s.astype(np.int32)

    plan = _Plan()
    plan.n_nodes = n_nodes
    plan.E = E
    plan.SHR = SHR
    plan.NG = NG
    plan.TBL = N_CORES * SHR
    plan.S_total = S_total
    plan.chunks = chunks
    plan.nodes_per_core = nodes_per_core
    plan.idx = idx
    plan.deg = deg
    plan.max_chunk_slots = max((g1 - g0) * D for (g0, g1, D, _) in chunks)
    plan.max_fold_slots = max((g1 - g0) * ((D + 1) // 2) for (g0, g1, D, _) in chunks)
    return plan


def build_inputs(plan, x, W1, b1, W2, b2, Wl, bl):
    """Per-core input dicts for run_bass_kernel_spmd."""
    SHR, NG = plan.SHR, plan.NG
    in_maps = []
    W1h = np.ascontiguousarray(W1.astype(np.float16))
    W2h = np.ascontiguousarray(W2.astype(np.float16))
    Wlh = np.ascontiguousarray(Wl.astype(np.float16))
    b1r = np.ascontiguousarray(np.broadcast_to(b1.astype(np.float16), (128, F_DIM)))
    b2r = np.ascontiguousarray(np.broadcast_to(b2.astype(np.float16), (128, F_DIM)))
    blr = np.ascontiguousarray(np.broadcast_to(bl.astype(np.float32), (128, C_DIM)))
    for c in range(N_CORES):
        nl = plan.nodes_per_core[c]
        n_c = len(nl)
        xT = np.zeros((F_DIM, SHR), dtype=np.float16)
        xT[:, :n_c] = x[nl].astype(np.float16).T
        degp1 = np.full((128, NG), 1e30, dtype=np.float32)
        dp = np.pad((plan.deg[nl] + 1).astype(np.float32), (0, SHR - n_c),
                    constant_values=1e30)
        degp1[:, :] = dp.reshape(NG, 128).T
        in_maps.append({
            "xT": xT,
            "degp1": degp1,
            "gidx": np.ascontiguousarray(plan.idx[c]),
            "W1": W1h, "W2": W2h, "Wl": Wlh,
            "b1r": b1r, "b2r": b2r, "blr": blr,
        })
    return in_maps


def build_bass(plan, repeat=1):
    SHR, NG, TBL, S_total = plan.SHR, plan.NG, plan.TBL, plan.S_total
    NF = NG * F_DIM
    nc = bass.Bass(num_devices=N_CORES, dynamic_dma_scratch_size=40960)

    xT_d = nc.dram_tensor("xT", [F_DIM, SHR], FP16, kind="ExternalInput")
    degp1_d = nc.dram_tensor("degp1", [128, NG], F32, kind="ExternalInput")
    gidx_d = nc.dram_tensor("gidx", [128, S_total], I32, kind="ExternalInput")
    W1_d = nc.dram_tensor("W1", [F_DIM, F_DIM], FP16, kind="ExternalInput")
    W2_d = nc.dram_tensor("W2", [F_DIM, F_DIM], FP16, kind="ExternalInput")
    Wl_d = nc.dram_tensor("Wl", [F_DIM, C_DIM], FP16, kind="ExternalInput")
    b1r_d = nc.dram_tensor("b1r", [128, F_DIM], FP16, kind="ExternalInput")
    b2r_d = nc.dram_tensor("b2r", [128, F_DIM], FP16, kind="ExternalInput")
    blr_d = nc.dram_tensor("blr", [128, C_DIM], F32, kind="ExternalInput")
    y_d = nc.dram_tensor("y", [SHR, C_DIM], F32, kind="ExternalOutput")

    gsh_all = [[nc.dram_tensor(f"gsh{i}_{r}", [SHR, F_DIM], FP16)
                for i in range(2)] for r in range(repeat)]
    gfull_all = [[nc.dram_tensor(f"gfull{i}_{r}", [TBL, F_DIM], FP16,
                                 addr_space="Shared")
                  for i in range(2)] for r in range(repeat)]

    rg = [list(range(N_CORES))]

    with tile.TileContext(nc) as tc:
        with (
            tc.tile_pool(name="const", bufs=1) as constp,
            tc.tile_pool(name="persist", bufs=1) as pers,
            tc.tile_pool(name="work", bufs=3) as workp,
            tc.tile_pool(name="xtb", bufs=1) as xtbp,
            tc.tile_pool(name="gath", bufs=2) as gathp,
            tc.tile_pool(name="fold", bufs=2) as foldp,
            tc.tile_pool(name="psum", bufs=2, space="PSUM") as psump,
        ):
            # ---- constants ----
            W1_sb = constp.tile([F_DIM, F_DIM], FP16, tag="W1")
            nc.sync.dma_start(out=W1_sb, in_=W1_d[:, :])
            W2_sb = constp.tile([F_DIM, F_DIM], FP16, tag="W2")
            nc.sync.dma_start(out=W2_sb, in_=W2_d[:, :])
            Wl_sb = constp.tile([F_DIM, C_DIM], FP16, tag="Wl")
            nc.sync.dma_start(out=Wl_sb, in_=Wl_d[:, :])
            b1_sb = constp.tile([128, F_DIM], FP16, tag="b1")
            nc.sync.dma_start(out=b1_sb, in_=b1r_d[:, :])
            b2_sb = constp.tile([128, F_DIM], FP16, tag="b2")
            nc.sync.dma_start(out=b2_sb, in_=b2r_d[:, :])
            bl_sb = constp.tile([128, C_DIM], F32, tag="bl")
            nc.sync.dma_start(out=bl_sb, in_=blr_d[:, :])
            ident = constp.tile([128, 128], FP16, tag="ident")
            make_identity(nc, ident[:, :])

            idx_sb = pers.tile([128, S_total], I32, tag="idx")
            nc.sync.dma_start(out=idx_sb, in_=gidx_d[:, :])

            # ---- dinv ----
            degp1_sb = constp.tile([128, NG], F32, tag="degp1")
            nc.sync.dma_start(out=degp1_sb, in_=degp1_d[:, :])
            rec_sb = constp.tile([128, NG], F32, tag="rec")
            nc.vector.reciprocal(rec_sb[:, :], degp1_sb[:, :])
            dinv_sb = constp.tile([128, NG], F32, tag="dinv")
            nc.scalar.activation(dinv_sb[:, :], rec_sb[:, :],
                                 mybir.ActivationFunctionType.Sqrt)
            # expanded fp16 replica: [128, NG*64], each group value repeated 64x
            dinv_rep = pers.tile([128, NF], FP16, tag="dinvrep")
            nc.vector.tensor_copy(
                dinv_rep[:, :].rearrange("p (g f) -> p g f", g=NG, f=F_DIM),
                dinv_sb[:, :].unsqueeze(2).broadcast_to([128, NG, F_DIM]),
            )

            def dense_matmul(lhsT_of_group, W_sb, n_out, out_tag, out_dtype=FP16):
                """out[p, g*n_out + f] = sum_k lhsT_g[k, p] * W[k, f]"""
                out_sb = pers.tile([128, NG * n_out], out_dtype, tag=out_tag)
                per_ps = max(1, 512 // n_out)
                for blk0 in range(0, NG, per_ps):
                    blk1 = min(NG, blk0 + per_ps)
                    w = (blk1 - blk0) * n_out
                    ps = psump.tile([128, 512], F32, tag="mmps")
                    for g in range(blk0, blk1):
                        k = g - blk0
                        nc.tensor.matmul(
                            ps[:, k * n_out:(k + 1) * n_out],
                            lhsT=lhsT_of_group(g),
                            rhs=W_sb[:, :],
                            start=True, stop=True,
                        )
                    nc.scalar.activation(out_sb[:, blk0 * n_out: blk0 * n_out + w],
                                         ps[:, :w],
                                         mybir.ActivationFunctionType.Copy)
                return out_sb

            def dense_matmul_nodemajor(h_sb, W_sb, n_out, out_tag,
                                       out_dtype=FP16):
                """Like dense_matmul but input is node-major [128, NG*64]:
                transposes 4 groups at a time via PE, then matmuls."""
                out_sb = pers.tile([128, NG * n_out], out_dtype, tag=out_tag)
                for blk0 in range(0, NG, 4):
                    blk1 = min(NG, blk0 + 4)
                    nb = blk1 - blk0
                    tps = psump.tile([64, 512], FP16, tag="tps")
                    for g in range(blk0, blk1):
                        k = g - blk0
                        nc.tensor.transpose(
                            tps[:, k * 128: k * 128 + 128],
                            in_=h_sb[:, g * F_DIM: (g + 1) * F_DIM],
                            identity=ident[:, :],
                        )
                    hTt = workp.tile([64, 512], FP16, tag="hTt", bufs=2)
                    nc.scalar.activation(hTt[:, :nb * 128], tps[:, :nb * 128],
                                         mybir.ActivationFunctionType.Copy)
                    ps = psump.tile([128, 512], F32, tag="mmps")
                    for g in range(blk0, blk1):
                        k = g - blk0
                        nc.tensor.matmul(
                            ps[:, k * n_out:(k + 1) * n_out],
                            lhsT=hTt[:, k * 128: k * 128 + 128],
                            rhs=W_sb[:, :],
                            start=True, stop=True,
                        )
                    nc.scalar.activation(
                        out_sb[:, blk0 * n_out: blk0 * n_out + nb * n_out],
                        ps[:, :nb * n_out],
                        mybir.ActivationFunctionType.Copy)
                return out_sb

            def aggregate(gfull_t, out_tag):
                """s[p, g*64+f] = sum_j gfull[idx[p, slot(g,j)], f]"""
                s_sb = pers.tile([128, NF], FP16, tag=out_tag)
                for (g0, g1, D, off) in plan.chunks:
                    G = g1 - g0
                    S = G * D
                    gt = gathp.tile([128, plan.max_chunk_slots * F_DIM], FP16,
                                    tag="gt")
                    # one indirect DMA per slot column ([128,1] offsets is
                    # the only indirect form that works on HW)
                    for s in range(S):
                        nc.gpsimd.indirect_dma_start(
                            out=gt[:, s * F_DIM:(s + 1) * F_DIM],
                            out_offset=None,
                            in_=gfull_t[:, :],
                            in_offset=IndirectOffsetOnAxis(
                                ap=idx_sb[:, off + s:off + s + 1], axis=0),
                        )
                    cur = gt
                    curD = D
                    cur_is_gt = True
                    while curD > 1:
                        half, odd = divmod(curD, 2)
                        newD = half + odd
                        if newD == 1:
                            dst = s_sb
                            dview = s_sb[:, g0 * F_DIM: g1 * F_DIM].rearrange(
                                "p (g d f) -> p g d f", g=G, d=1, f=F_DIM)
                        else:
                            dst = foldp.tile(
                                [128, plan.max_fold_slots * F_DIM],
                                FP16, tag="fold")
                            dview = dst[:, :G * newD * F_DIM].rearrange(
                                "p (g d f) -> p g d f", g=G, d=newD, f=F_DIM)
                        cview = (cur[:, :G * curD * F_DIM] if cur_is_gt
                                 else cur[:, :G * curD * F_DIM]).rearrange(
                            "p (g d f) -> p g d f", g=G, d=curD, f=F_DIM)
                        nc.vector.tensor_tensor(
                            dview[:, :, :half, :],
                            cview[:, :, 0:2 * half:2, :],
                            cview[:, :, 1:2 * half:2, :],
                            op=mybir.AluOpType.add,
                        )
                        if odd:
                            nc.vector.tensor_copy(
                                dview[:, :, half:half + 1, :],
                                cview[:, :, curD - 1:curD, :],
                            )
                        cur = dst
                        curD = newD
                        cur_is_gt = False
                    if D == 1:
                        nc.vector.tensor_copy(
                            s_sb[:, g0 * F_DIM: g1 * F_DIM],
                            gt[:, :S * F_DIM],
                        )
                return s_sb

            def mul_rep(src_sb, tag, bufs=1):
                o = workp.tile([128, NF], FP16, tag=tag, bufs=bufs)
                nc.vector.tensor_tensor(o[:, :NF], src_sb[:, :NF],
                                        dinv_rep[:, :NF],
                                        op=mybir.AluOpType.mult)
                return o

            def finalize(s_sb, g_sb, b_sb, out_tag):
                """relu(dinv*(s + g) + b) -> fp16 [128, NF]
                (g = dinv*h is the same table row data sent to the AllGather,
                so dinv*g = dinv^2*h, the self-loop term)."""
                tA = workp.tile([128, NF], FP16, tag="finA", bufs=1)
                nc.vector.tensor_tensor(tA[:, :], s_sb[:, :], g_sb[:, :],
                                        op=mybir.AluOpType.add)
                tB = workp.tile([128, NF], FP16, tag="finB", bufs=1)
                nc.vector.tensor_tensor(tB[:, :], tA[:, :], dinv_rep[:, :],
                                        op=mybir.AluOpType.mult)
                tC = workp.tile([128, NF], FP16, tag="finA", bufs=1)
                nc.vector.tensor_tensor(
                    tC[:, :].rearrange("p (g f) -> p g f", g=NG, f=F_DIM),
                    tB[:, :].rearrange("p (g f) -> p g f", g=NG, f=F_DIM),
                    b_sb[:, :].unsqueeze(1).broadcast_to([128, NG, F_DIM]),
                    op=mybir.AluOpType.add,
                )
                act = pers.tile([128, NF], FP16, tag=out_tag)
                nc.scalar.activation(act[:, :], tC[:, :],
                                     mybir.ActivationFunctionType.Relu)
                return act

            def cc_observe(gfull_t):
                # Tiny SWDGE read of the AllGather output: carries the
                # collective-done wait once, so the real gathers (same
                # engine, later in FIFO) don't each need that sem slot
                # (HWDGE/SWDGE DMA instructions fit only 2 sync waits).
                obs = constp.tile([1, F_DIM], FP16, tag="ccobs")
                nc.gpsimd.dma_start(out=obs[:, :], in_=gfull_t[0:1, :])

            def pipeline(gsh, gfull):
                # ================= layer 1 =================
                xT_sb = xtbp.tile([F_DIM, SHR], FP16, tag="xtb")
                nc.sync.dma_start(out=xT_sb, in_=xT_d[:, :])

                def xT_lhsT(g):
                    return xT_sb[:, g * 128: g * 128 + 128]

                h1pre = dense_matmul(xT_lhsT, W1_sb, F_DIM, "hpre")
                g1_sb = mul_rep(h1pre, tag="gsb")
                nc.sync.dma_start(
                    out=gsh[0][:, :].rearrange("(p g) f -> p (g f)", p=128),
                    in_=g1_sb[:, :],
                )
                nc.gpsimd.collective_compute(
                    "AllGather", mybir.AluOpType.bypass, replica_groups=rg,
                    ins=[gsh[0][:, :].opt()], outs=[gfull[0][:, :].opt()],
                )
                cc_observe(gfull[0])
                s1 = aggregate(gfull[0], "s")
                h1 = finalize(s1, g1_sb, b1_sb, "h1")

                # ================= layer 2 =================
                h2pre = dense_matmul_nodemajor(h1, W2_sb, F_DIM, "hpre")
                g2_sb = mul_rep(h2pre, tag="gsb")
                nc.sync.dma_start(
                    out=gsh[1][:, :].rearrange("(p g) f -> p (g f)", p=128),
                    in_=g2_sb[:, :],
                )
                nc.gpsimd.collective_compute(
                    "AllGather", mybir.AluOpType.bypass, replica_groups=rg,
                    ins=[gsh[1][:, :].opt()], outs=[gfull[1][:, :].opt()],
                )
                cc_observe(gfull[1])
                s2 = aggregate(gfull[1], "s")
                h2a = finalize(s2, g2_sb, b2_sb, "s")
                h2 = pers.tile([128, NF], FP16, tag="hpre")
                nc.vector.tensor_tensor(h2[:, :], h2a[:, :], h1[:, :],
                                        op=mybir.AluOpType.add)

                # ================= head + log_softmax =================
                y_sb = dense_matmul_nodemajor(h2, Wl_sb, C_DIM, "ysb", F32)
                NC_ = NG * C_DIM
                yb = workp.tile([128, NC_], F32, tag="lsm")
                nc.vector.tensor_tensor(
                    yb[:, :].rearrange("p (g f) -> p g f", g=NG, f=C_DIM),
                    y_sb[:, :].rearrange("p (g f) -> p g f", g=NG, f=C_DIM),
                    bl_sb[:, :].unsqueeze(1).broadcast_to([128, NG, C_DIM]),
                    op=mybir.AluOpType.add,
                )
                rmax = workp.tile([128, NG], F32, tag="red")
                nc.vector.tensor_reduce(
                    rmax[:, :],
                    yb[:, :].rearrange("p (g f) -> p g f", g=NG, f=C_DIM),
                    axis=mybir.AxisListType.X, op=mybir.AluOpType.max,
                )
                tsub = workp.tile([128, NC_], F32, tag="lsm")
                nc.vector.tensor_tensor(
                    tsub[:, :].rearrange("p (g f) -> p g f", g=NG, f=C_DIM),
                    yb[:, :].rearrange("p (g f) -> p g f", g=NG, f=C_DIM),
                    rmax[:, :].unsqueeze(2).broadcast_to([128, NG, C_DIM]),
                    op=mybir.AluOpType.subtract,
                )
                e_sb = workp.tile([128, NC_], F32, tag="lsm")
                nc.scalar.activation(e_sb[:, :], tsub[:, :],
                                     mybir.ActivationFunctionType.Exp)
                ssum = workp.tile([128, NG], F32, tag="red")
                nc.vector.tensor_reduce(
                    ssum[:, :],
                    e_sb[:, :].rearrange("p (g f) -> p g f", g=NG, f=C_DIM),
                    axis=mybir.AxisListType.X, op=mybir.AluOpType.add,
                )
                lse = workp.tile([128, NG], F32, tag="red")
                nc.scalar.activation(lse[:, :], ssum[:, :],
                                     mybir.ActivationFunctionType.Ln)
                yout = workp.tile([128, NC_], F32, tag="lsm")
                nc.vector.tensor_tensor(
                    yout[:, :].rearrange("p (g f) -> p g f", g=NG, f=C_DIM),
                    tsub[:, :].rearrange("p (g f) -> p g f", g=NG, f=C_DIM),
                    lse[:, :].unsqueeze(2).broadcast_to([128, NG, C_DIM]),
                    op=mybir.AluOpType.subtract,
                )
                nc.sync.dma_start(
                    out=y_d[:, :].rearrange("(p g) f -> p (g f)", p=128),
                    in_=yout[:, :],
                )

            for r in range(repeat):
                pipeline(gsh_all[r], gfull_all[r])

    if SPLIT_WAITS:
        split_excess_waits(nc)
    return nc


SPLIT_WAITS = True  # set False for MultiCoreSim (race detector needs inst_map)

# walrus codegen rejects instructions whose sync_info carries more waits
# than the ISA struct has slots for ("Too many sync wait commands").  Tile
# packs minimal waits but does not know the per-struct caps, so split any
# excess into standalone EventSemaphore (sequencer wait) instructions just
# before the over-subscribed instruction on the same engine.
_WAIT_CAPS = {}
_WAIT_CAP_DEFAULT = 1
_WAIT_CAP_SKIP = {"EventSemaphore", "CollectiveCompute",
                  "AllEngineBarrier", "RegisterMove", "UnconditionalBranch"}


def split_excess_waits(nc):
    n_split = 0
    for fn in nc.m.functions:
        for blk in fn.blocks:
            out = []
            for ins in blk.instructions:
                si = ins.sync_info
                op = ins.opcode
                if (si is not None and op not in _WAIT_CAP_SKIP
                        and si.on_wait is not None):
                    cap = _WAIT_CAPS.get(op, _WAIT_CAP_DEFAULT)
                    waits = list(si.on_wait)
                    if len(waits) > cap:
                        keep = waits[-cap:] if cap else []
                        for k, w in enumerate(waits[:len(waits) - cap]):
                            ev = mybir.InstEventSemaphore(
                                name=f"{ins.name}-ws{k}", ins=[], outs=[])
                            ev.engine = ins.engine
                            ev.sync_info = mybir.SyncInfo(on_wait=[w],
                                                          on_update=[])
                            out.append(ev)
                            n_split += 1
                        si.on_wait = keep
                out.append(ins)
            blk.instructions = out
    return n_split


_CACHE = {}

LAST_RESULT = None


def kernel(x, edge_index, W1, b1, W2, b2, Wl, bl):
    global LAST_RESULT
    x = np.asarray(x)
    edge_index = np.asarray(edge_index)
    n_nodes = x.shape[0]
    key = (n_nodes, edge_index.shape[1], bytes(np.asarray(edge_index[1, :64]).astype(np.int64)))
    if key not in _CACHE:
        plan = build_plan(edge_index, n_nodes)
        nc = build_bass(plan)
        _CACHE[key] = (plan, nc)
    plan, nc = _CACHE[key]

    in_maps = build_inputs(plan, x, np.asarray(W1), np.asarray(b1),
                           np.asarray(W2), np.asarray(b2),
                           np.asarray(Wl), np.asarray(bl))
    res = run_bass_kernel_spmd(nc, in_maps, core_ids=list(range(N_CORES)),
                               trace=False)
    LAST_RESULT = res
    y = np.empty((n_nodes, C_DIM), dtype=np.float32)
    parts = []
    for c in range(N_CORES):
        n_c = len(plan.nodes_per_core[c])
        yc = res.results[c]["y"]  # row p*NG+g -> rank g*128+p
        yc = yc.reshape(128, plan.NG, C_DIM).transpose(1, 0, 2).reshape(plan.SHR, C_DIM)
        parts.append(yc[:n_c])
    y[np.concatenate(plan.nodes_per_core)] = np.concatenate(parts, axis=0)
    return y



# revision 10
# speedup vs baseline: 1.0159x; 1.0159x over previous
"""GCN (2-layer + linear head + log_softmax) on 8 Trainium2 NeuronCores.

Strategy (graph/data parallel, per sharding hint):
  - Nodes partitioned across 8 cores (degree-sorted serpentine), weights
    replicated.  Per GCN layer each core computes h = x_shard @ W on PE,
    scales rows by dinv = (deg+1)^-1/2 and AllGathers the scaled table
    (fp16, feature-duplicated to 256B rows) in 4 window chunks.
  - Aggregation: batched SWDGE dma_gather (Ant extended instruction, mlp
    gpsimd library) pulls per-edge source rows (256B granules, int16
    window-local indices) into SBUF column tiles; PE one-hot "segment
    matmuls" (M[e, rank] = [segid[e] == rank]) reduce each 128-slot column
    into per-destination partial sums accumulated in PSUM; DVE folds the
    4 window partials into s.  This replaces the per-slot indirect-DMA
    gathers of the old kernel (~1.6us/128 rows) with ~1.4ns/row batched
    transfers.
  - Math identity:  out[d] = dinv[d] * sum_{e:dst=d} dinv[src]*h[src]
                             + dinv[d]^2 * h[d] + b
    so the gathered table is pre-scaled by dinv and no per-edge
    coefficients are needed.

Host-side numpy does only graph-structure preprocessing (degree counting,
node->core/rank assignment, gather-index/segment-id construction) and the
output unpermute.  All floating-point tensor math runs on the NeuronCores.
"""

import os

import numpy as np

import concourse.bass as bass
import concourse.bacc as bacc
import concourse.mybir as mybir
import concourse.tile as tile
from concourse.bass_utils import run_bass_kernel_spmd
from concourse.masks import make_identity
from concourse import library_config

FP16 = mybir.dt.float16
F32 = mybir.dt.float32
I16 = mybir.dt.int16

N_CORES = 8
P = 128           # partitions
F_DIM = 64        # in = hidden = 64
C_DIM = 16
NG = 98           # feature groups per core (SHR / 128)
SHR = 12544       # node slots per core (128 * 98)
W = 4             # gather windows (int16 index range)
WP = 32           # partitions per window (128 / 4)
WG = WP * NG      # rows contributed per core per window = 3136
WROWS = N_CORES * WG   # rows per window table = 25088
RG = 8            # dst groups per PSUM range
NR = (NG + RG - 1) // RG   # 13 ranges
CAP = 48          # max gather columns per dma_gather call
ZROW = WP * NG - 1         # window-local zero row: c=0, p_local=31, g=97 -> 3135
SEG_PAD = 200.0   # segid sentinel for padding slots (matches no rank)

# reserved ranks (one zero table row per window): rank = 128*97 + (32w+31)
RESERVED = tuple(128 * (NG - 1) + (WP * w + WP - 1) for w in range(W))


class _Call:
    __slots__ = ("w", "ncols", "off16", "col_off", "cols")


class _Plan:
    pass


def build_plan(edge_index, n_nodes):
    """Host-side graph preprocessing.  Pure index math, O(E log E)."""
    src = np.asarray(edge_index[0], dtype=np.int64)
    dst = np.asarray(edge_index[1], dtype=np.int64)
    E = src.shape[0]

    deg = np.bincount(dst, minlength=n_nodes).astype(np.int64)

    # serpentine degree-desc core assignment
    order = np.argsort(-deg, kind="stable")
    pos = np.arange(n_nodes)
    blk, lane = pos // N_CORES, pos % N_CORES
    core_of_pos = np.where(blk % 2 == 0, lane, N_CORES - 1 - lane)
    nodes_per_core = [order[core_of_pos == c] for c in range(N_CORES)]
    n_max = max(len(v) for v in nodes_per_core)
    assert n_max <= SHR - W, "shard overflow"

    resv = set(RESERVED)
    usable = np.array([r for r in range(SHR) if r not in resv], dtype=np.int64)

    rank_of = np.full(n_nodes, -1, dtype=np.int64)
    core_of = np.full(n_nodes, -1, dtype=np.int64)
    for c, nl in enumerate(nodes_per_core):
        rank_of[nl] = usable[: len(nl)]
        core_of[nl] = c

    p_of = rank_of % P
    g_of = rank_of // P
    w_of = p_of // WP
    # window-local table row of a node
    loc_of = core_of * WG + (p_of % WP) * NG + g_of

    # per-core per (dst-group, src-window) edge counts
    cnt = np.zeros((N_CORES, NG, W), dtype=np.int64)
    ecore = core_of[dst]
    for c in range(N_CORES):
        m = ecore == c
        np.add.at(cnt[c], (g_of[dst[m]], w_of[src[m]]), 1)

    # shared (max over cores) column counts per (g, w); >=1 so every group
    # appears in window 0 (s init via copy) and DVE accum stays coarse
    cols_gw = np.maximum(1, -(-cnt.max(axis=0) // P))   # [NG, W]

    # global column offsets, order (w, range, g)
    o_gw = np.zeros((NG, W), dtype=np.int64)
    col = 0
    rw_list = []   # (w, R, rgw, [calls])
    for w in range(W):
        for R in range(NR):
            g0, g1 = R * RG, min(NG, R * RG + RG)
            span0 = col
            for g in range(g0, g1):
                o_gw[g, w] = col
                col += int(cols_gw[g, w])
            # split span into calls of <= CAP columns
            calls = []
            c0 = span0
            while c0 < col:
                c1 = min(col, c0 + CAP)
                call = _Call()
                call.w = w
                call.ncols = c1 - c0
                call.off16 = c0 * 8          # slot offset / 16
                call.col_off = c0
                call.cols = []
                calls.append(call)
                c0 = c1
            # per-column matmul descriptors
            for g in range(g0, g1):
                for k in range(int(cols_gw[g, w])):
                    cg = int(o_gw[g, w]) + k
                    for call in calls:
                        if call.col_off <= cg < call.col_off + call.ncols:
                            call.cols.append(
                                (cg - call.col_off, g - g0,
                                 k == 0, k == int(cols_gw[g, w]) - 1))
                            break
            rw_list.append((w, R, g1 - g0, calls))
    TOT_COLS = col
    TOT_SLOTS = TOT_COLS * P
    maxcall = max(c.ncols for (_, _, _, cl) in rw_list for c in cl)

    # per-core gather indices + segment ids
    idx16 = np.empty((N_CORES, P, TOT_SLOTS // 16), dtype=np.int16)
    segid = np.empty((N_CORES, P, TOT_COLS), dtype=np.float16)
    for c in range(N_CORES):
        m = ecore == c
        d_g = g_of[dst[m]]
        d_p = p_of[dst[m]]
        s_w = w_of[src[m]]
        s_loc = loc_of[src[m]]
        key = d_g * W + s_w
        o = np.argsort(key, kind="stable")
        key_s = key[o]
        first = np.searchsorted(key_s, key_s, side="left")
        k = np.arange(len(key_s)) - first
        colno = o_gw[d_g[o], s_w[o]] + k // P
        part = k % P
        slot = colno * P + part
        idx_flat = np.full(TOT_SLOTS, ZROW, dtype=np.int16)
        seg_flat = np.full(TOT_SLOTS, SEG_PAD, dtype=np.float16)
        idx_flat[slot] = s_loc[o].astype(np.int16)
        seg_flat[slot] = d_p[o].astype(np.float16)
        wrapped = idx_flat.reshape(TOT_SLOTS // 16, 16).T   # [16, S/16]
        idx16[c] = np.tile(wrapped, (8, 1))
        segid[c] = seg_flat.reshape(TOT_COLS, P).T

    plan = _Plan()
    plan.n_nodes = n_nodes
    plan.E = E
    plan.TOT_COLS = TOT_COLS
    plan.TOT_SLOTS = TOT_SLOTS
    plan.maxcall = maxcall
    plan.rw_list = rw_list
    plan.nodes_per_core = nodes_per_core
    plan.rank_of = rank_of
    plan.usable = usable
    plan.idx16 = idx16
    plan.segid = segid
    plan.deg = deg
    return plan


def build_inputs(plan, x, W1, b1, W2, b2, Wl, bl):
    """Per-core input dicts for run_bass_kernel_spmd."""
    in_maps = []
    W1h = np.ascontiguousarray(W1.astype(np.float16))
    W2h = np.ascontiguousarray(W2.astype(np.float16))
    Wlh = np.ascontiguousarray(Wl.astype(np.float16))
    b1r = np.ascontiguousarray(np.broadcast_to(b1.astype(np.float16), (P, F_DIM)))
    b2r = np.ascontiguousarray(np.broadcast_to(b2.astype(np.float16), (P, F_DIM)))
    blr = np.ascontiguousarray(np.broadcast_to(bl.astype(np.float32), (P, C_DIM)))
    iota = np.ascontiguousarray(
        np.broadcast_to(np.arange(P, dtype=np.float16), (P, P)))
    for c in range(N_CORES):
        nl = plan.nodes_per_core[c]
        ranks = plan.usable[: len(nl)]
        xT = np.zeros((F_DIM, SHR), dtype=np.float16)
        xT[:, ranks] = np.asarray(x)[nl].astype(np.float16).T
        degp1 = np.full((P, NG), 1e30, dtype=np.float32)
        degp1[ranks % P, ranks // P] = (plan.deg[nl] + 1).astype(np.float32)
        in_maps.append({
            "xT": xT,
            "degp1": degp1,
            "gidx": np.ascontiguousarray(plan.idx16[c]),
            "segid": np.ascontiguousarray(plan.segid[c]),
            "iota": iota,
            "W1": W1h, "W2": W2h, "Wl": Wlh,
            "b1r": b1r, "b2r": b2r, "blr": blr,
        })
    return in_maps


def build_bass(plan, repeat=1):
    NF = NG * F_DIM
    TOT16 = plan.TOT_SLOTS // 16
    MC = plan.maxcall
    nc = bacc.Bacc("TRN2", target_bir_lowering=False, debug=False,
                   num_devices=N_CORES, dynamic_dma_scratch_size=16384)

    xT_d = nc.dram_tensor("xT", [F_DIM, SHR], FP16, kind="ExternalInput")
    degp1_d = nc.dram_tensor("degp1", [P, NG], F32, kind="ExternalInput")
    gidx_d = nc.dram_tensor("gidx", [P, TOT16], I16, kind="ExternalInput")
    segid_d = nc.dram_tensor("segid", [P, plan.TOT_COLS], FP16,
                             kind="ExternalInput")
    iota_d = nc.dram_tensor("iota", [P, P], FP16, kind="ExternalInput")
    W1_d = nc.dram_tensor("W1", [F_DIM, F_DIM], FP16, kind="ExternalInput")
    W2_d = nc.dram_tensor("W2", [F_DIM, F_DIM], FP16, kind="ExternalInput")
    Wl_d = nc.dram_tensor("Wl", [F_DIM, C_DIM], FP16, kind="ExternalInput")
    b1r_d = nc.dram_tensor("b1r", [P, F_DIM], FP16, kind="ExternalInput")
    b2r_d = nc.dram_tensor("b2r", [P, F_DIM], FP16, kind="ExternalInput")
    blr_d = nc.dram_tensor("blr", [P, C_DIM], F32, kind="ExternalInput")
    y_d = nc.dram_tensor("y", [SHR, C_DIM], F32, kind="ExternalOutput")

    gsh_all = [[[nc.dram_tensor(f"gsh{l}_{w}_{r}", [WG, 2 * F_DIM], FP16)
                 for w in range(W)] for l in range(2)] for r in range(repeat)]
    gfull_all = [[[nc.dram_tensor(f"gfull{l}_{w}_{r}", [WROWS, 2 * F_DIM],
                                  FP16, addr_space="Shared")
                   for w in range(W)] for l in range(2)] for r in range(repeat)]

    rg = [list(range(N_CORES))]

    with tile.TileContext(nc) as tc:
        with (
            tc.tile_pool(name="const", bufs=1) as constp,
            tc.tile_pool(name="persist", bufs=1) as pers,
            tc.tile_pool(name="work", bufs=2) as workp,
            tc.tile_pool(name="xt", bufs=2) as xtp,
            tc.tile_pool(name="gath", bufs=2) as gathp,
            tc.tile_pool(name="mm", bufs=2) as mp,
            tc.tile_pool(name="psum", bufs=2, space="PSUM") as psump,
        ):
            # Load the mlp gpsimd library (dma_gather) up front: a reload
            # auto-inserted mid-program next to in-flight collectives kills
            # the Q7 cores (NRT_EXEC_UNIT_UNRECOVERABLE).
            nc.gpsimd.load_library(library_config.mlp)

            # ---- constants ----
            W1_sb = constp.tile([F_DIM, F_DIM], FP16, tag="W1")
            nc.sync.dma_start(out=W1_sb, in_=W1_d[:, :])
            W2_sb = constp.tile([F_DIM, F_DIM], FP16, tag="W2")
            nc.sync.dma_start(out=W2_sb, in_=W2_d[:, :])
            Wl_sb = constp.tile([F_DIM, C_DIM], FP16, tag="Wl")
            nc.sync.dma_start(out=Wl_sb, in_=Wl_d[:, :])
            b1_sb = constp.tile([P, F_DIM], FP16, tag="b1")
            nc.sync.dma_start(out=b1_sb, in_=b1r_d[:, :])
            b2_sb = constp.tile([P, F_DIM], FP16, tag="b2")
            nc.sync.dma_start(out=b2_sb, in_=b2r_d[:, :])
            bl_sb = constp.tile([P, C_DIM], F32, tag="bl")
            nc.sync.dma_start(out=bl_sb, in_=blr_d[:, :])
            ident = constp.tile([P, P], FP16, tag="ident")
            make_identity(nc, ident[:, :])
            iota_sb = constp.tile([P, P], FP16, tag="iota")
            nc.sync.dma_start(out=iota_sb, in_=iota_d[:, :])

            idx_sb = constp.tile([P, TOT16], I16, tag="idx")
            nc.sync.dma_start(out=idx_sb, in_=gidx_d[:, :])
            segid_sb = constp.tile([P, plan.TOT_COLS], FP16, tag="segid")
            nc.sync.dma_start(out=segid_sb, in_=segid_d[:, :])

            # ---- dinv ----
            degp1_sb = constp.tile([P, NG], F32, tag="degp1")
            nc.sync.dma_start(out=degp1_sb, in_=degp1_d[:, :])
            rec_sb = constp.tile([P, NG], F32, tag="rec")
            nc.vector.reciprocal(rec_sb[:, :], degp1_sb[:, :])
            dinv_sb = constp.tile([P, NG], F32, tag="dinv")
            nc.scalar.activation(dinv_sb[:, :], rec_sb[:, :],
                                 mybir.ActivationFunctionType.Sqrt)
            dinv_rep = constp.tile([P, NF], FP16, tag="dinvrep")
            nc.vector.tensor_copy(
                dinv_rep[:, :].rearrange("p (g f) -> p g f", g=NG, f=F_DIM),
                dinv_sb[:, :].unsqueeze(2).broadcast_to([P, NG, F_DIM]),
            )

            def dense_matmul_stream(W_sb, out_tag):
                """layer-1 shard matmul, xT streamed from DRAM."""
                out_sb = pers.tile([P, NF], FP16, tag=out_tag)
                per_ps = 8
                for blk0 in range(0, NG, per_ps):
                    blk1 = min(NG, blk0 + per_ps)
                    nb = blk1 - blk0
                    xt = xtp.tile([F_DIM, per_ps * P], FP16, tag="xt")
                    nc.sync.dma_start(out=xt[:, :nb * P],
                                      in_=xT_d[:, blk0 * P: blk1 * P])
                    ps = psump.tile([P, 512], F32, tag="mmps")
                    for g in range(blk0, blk1):
                        kk = g - blk0
                        nc.tensor.matmul(
                            ps[:, kk * F_DIM:(kk + 1) * F_DIM],
                            lhsT=xt[:, kk * P: kk * P + P],
                            rhs=W_sb[:, :], start=True, stop=True)
                    nc.scalar.activation(
                        out_sb[:, blk0 * F_DIM: blk0 * F_DIM + nb * F_DIM],
                        ps[:, :nb * F_DIM],
                        mybir.ActivationFunctionType.Copy)
                return out_sb

            def dense_matmul_nodemajor(h_sb, W_sb, n_out, out_tag,
                                       out_dtype=FP16):
                """input node-major [128, NG*64]: PE-transpose 4 groups at a
                time, then matmul."""
                out_sb = pers.tile([P, NG * n_out], out_dtype, tag=out_tag)
                for blk0 in range(0, NG, 4):
                    blk1 = min(NG, blk0 + 4)
                    nb = blk1 - blk0
                    tps = psump.tile([F_DIM, 512], FP16, tag="tps")
                    for g in range(blk0, blk1):
                        kk = g - blk0
                        nc.tensor.transpose(
                            tps[:, kk * P: kk * P + P],
                            in_=h_sb[:, g * F_DIM: (g + 1) * F_DIM],
                            identity=ident[:, :],
                        )
                    hTt = workp.tile([F_DIM, 512], FP16, tag="hTt")
                    nc.scalar.activation(hTt[:, :nb * P], tps[:, :nb * P],
                                         mybir.ActivationFunctionType.Copy)
                    ps = psump.tile([P, 512], F32, tag="mmps")
                    for g in range(blk0, blk1):
                        kk = g - blk0
                        nc.tensor.matmul(
                            ps[:, kk * n_out:(kk + 1) * n_out],
                            lhsT=hTt[:, kk * P: kk * P + P],
                            rhs=W_sb[:, :], start=True, stop=True)
                    nc.scalar.activation(
                        out_sb[:, blk0 * n_out: blk0 * n_out + nb * n_out],
                        ps[:, :nb * n_out],
                        mybir.ActivationFunctionType.Copy)
                return out_sb

            def mul_rep(h_sb, tag):
                o = pers.tile([P, NF], FP16, tag=tag)
                nc.vector.tensor_tensor(o[:, :], h_sb[:, :], dinv_rep[:, :],
                                        op=mybir.AluOpType.mult)
                return o

            def cc_observe(gfull_t):
                # tiny SWDGE read carries the collective-done wait once
                obs = constp.tile([1, F_DIM], FP16, tag="ccobs")
                nc.gpsimd.dma_start(out=obs[:, :], in_=gfull_t[0:1, 0:F_DIM])

            def table_exchange(g_sb, gsh, gfull):
                """write dinv-scaled shard (duplicated to 256B rows) and
                AllGather per window."""
                for w in range(W):
                    gv = gsh[w][:, :].rearrange("(p g) f -> p g f", p=WP, g=NG)
                    src3 = g_sb[WP * w: WP * (w + 1), :].rearrange(
                        "p (g f) -> p g f", g=NG, f=F_DIM)
                    nc.sync.dma_start(out=gv[:, :, 0:F_DIM], in_=src3)
                    nc.sync.dma_start(out=gv[:, :, F_DIM:2 * F_DIM], in_=src3)
                    nc.gpsimd.collective_compute(
                        "AllGather", mybir.AluOpType.bypass,
                        replica_groups=rg,
                        ins=[gsh[w][:, :].opt()], outs=[gfull[w][:, :].opt()],
                    )
                    cc_observe(gfull[w])

            AGG_MODE = int(os.environ.get("KAGG", "3"))

            def aggregate(gfull, out_tag):
                s_sb = pers.tile([P, NF], FP16, tag=out_tag)
                if AGG_MODE == 0:
                    nc.vector.memset(s_sb[:, :], 0.0)
                    return s_sb
                for (w, R, rgw, calls) in plan.rw_list:
                    if AGG_MODE >= 3:
                        ps = psump.tile([P, 512], F32, tag="agg")
                    for call in calls:
                        ncol = call.ncols
                        gt = gathp.tile([P, MC * P], FP16, tag="gt")
                        nc.gpsimd.dma_gather(
                            out_ap=gt[:, :ncol * P].rearrange(
                                "p (c e) -> p c e", c=ncol, e=P),
                            in_ap=gfull[w][:, :],
                            idxs_ap=idx_sb[:, call.off16: call.off16 + ncol * 8],
                            num_idxs=ncol * P,
                            num_idxs_reg=ncol * P,
                            elem_size=P,
                            single_packet=False,
                        )
                        if AGG_MODE < 2:
                            nc.vector.tensor_copy(
                                s_sb[:, R * F_DIM: R * F_DIM + F_DIM],
                                gt[:, 0:F_DIM])
                            continue
                        mt = mp.tile([P, MC * P], FP16, tag="mt")
                        nc.vector.tensor_tensor(
                            mt[:, :ncol * P].rearrange(
                                "p (c k) -> p c k", c=ncol, k=P),
                            segid_sb[:, call.col_off: call.col_off + ncol]
                            .unsqueeze(2).broadcast_to([P, ncol, P]),
                            iota_sb[:, :].unsqueeze(1).broadcast_to(
                                [P, ncol, P]),
                            op=mybir.AluOpType.is_equal,
                        )
                        if AGG_MODE < 3:
                            nc.vector.tensor_copy(
                                s_sb[:, R * F_DIM: R * F_DIM + F_DIM],
                                mt[:, 0:F_DIM])
                            continue
                        for (ci, gl, st, sp) in call.cols:
                            nc.tensor.matmul(
                                ps[:, gl * F_DIM:(gl + 1) * F_DIM],
                                lhsT=mt[:, ci * P: ci * P + P],
                                rhs=gt[:, ci * P: ci * P + F_DIM],
                                start=st, stop=sp)
                    if AGG_MODE < 3:
                        continue
                    sl = s_sb[:, R * RG * F_DIM: R * RG * F_DIM + rgw * F_DIM]
                    if w == 0:
                        nc.vector.tensor_copy(sl, ps[:, :rgw * F_DIM])
                    else:
                        nc.vector.tensor_tensor(sl, sl, ps[:, :rgw * F_DIM],
                                                op=mybir.AluOpType.add)
                if AGG_MODE < 3:
                    nc.vector.memset(s_sb[:, :], 0.0)
                return s_sb

            def finalize(s_sb, g_sb, b_sb, out_tag):
                """relu(dinv*(s + g) + b); destroys s_sb and g_sb."""
                nc.vector.tensor_tensor(s_sb[:, :], s_sb[:, :], g_sb[:, :],
                                        op=mybir.AluOpType.add)
                nc.vector.tensor_tensor(g_sb[:, :], s_sb[:, :],
                                        dinv_rep[:, :],
                                        op=mybir.AluOpType.mult)
                nc.vector.tensor_tensor(
                    s_sb[:, :].rearrange("p (g f) -> p g f", g=NG, f=F_DIM),
                    g_sb[:, :].rearrange("p (g f) -> p g f", g=NG, f=F_DIM),
                    b_sb[:, :].unsqueeze(1).broadcast_to([P, NG, F_DIM]),
                    op=mybir.AluOpType.add,
                )
                act = pers.tile([P, NF], FP16, tag=out_tag)
                nc.scalar.activation(act[:, :], s_sb[:, :],
                                     mybir.ActivationFunctionType.Relu)
                return act

            def pipeline(gsh2, gfull2):
                # ================= layer 1 =================
                h1pre = dense_matmul_stream(W1_sb, "hpre")
                g1_sb = mul_rep(h1pre, "gsb")
                table_exchange(g1_sb, gsh2[0], gfull2[0])
                s1 = aggregate(gfull2[0], "s")
                h1 = finalize(s1, g1_sb, b1_sb, "h1")

                # ================= layer 2 =================
                h2pre = dense_matmul_nodemajor(h1, W2_sb, F_DIM, "hpre")
                g2_sb = mul_rep(h2pre, "gsb")
                table_exchange(g2_sb, gsh2[1], gfull2[1])
                s2 = aggregate(gfull2[1], "s")
                h2a = finalize(s2, g2_sb, b2_sb, "gsb")
                h2 = pers.tile([P, NF], FP16, tag="hpre")
                nc.vector.tensor_tensor(h2[:, :], h2a[:, :], h1[:, :],
                                        op=mybir.AluOpType.add)

                # ================= head + log_softmax =================
                y_sb = dense_matmul_nodemajor(h2, Wl_sb, C_DIM, "ysb", F32)
                NC_ = NG * C_DIM
                yb = workp.tile([P, NC_], F32, tag="lsm", bufs=3)
                nc.vector.tensor_tensor(
                    yb[:, :].rearrange("p (g f) -> p g f", g=NG, f=C_DIM),
                    y_sb[:, :].rearrange("p (g f) -> p g f", g=NG, f=C_DIM),
                    bl_sb[:, :].unsqueeze(1).broadcast_to([P, NG, C_DIM]),
                    op=mybir.AluOpType.add,
                )
                rmax = workp.tile([P, NG], F32, tag="red", bufs=3)
                nc.vector.tensor_reduce(
                    rmax[:, :],
                    yb[:, :].rearrange("p (g f) -> p g f", g=NG, f=C_DIM),
                    axis=mybir.AxisListType.X, op=mybir.AluOpType.max,
                )
                tsub = workp.tile([P, NC_], F32, tag="lsm", bufs=3)
                nc.vector.tensor_tensor(
                    tsub[:, :].rearrange("p (g f) -> p g f", g=NG, f=C_DIM),
                    yb[:, :].rearrange("p (g f) -> p g f", g=NG, f=C_DIM),
                    rmax[:, :].unsqueeze(2).broadcast_to([P, NG, C_DIM]),
                    op=mybir.AluOpType.subtract,
                )
                e_sb = workp.tile([P, NC_], F32, tag="lsm", bufs=3)
                nc.scalar.activation(e_sb[:, :], tsub[:, :],
                                     mybir.ActivationFunctionType.Exp)
                ssum = workp.tile([P, NG], F32, tag="red", bufs=3)
                nc.vector.tensor_reduce(
                    ssum[:, :],
                    e_sb[:, :].rearrange("p (g f) -> p g f", g=NG, f=C_DIM),
                    axis=mybir.AxisListType.X, op=mybir.AluOpType.add,
                )
                lse = workp.tile([P, NG], F32, tag="red", bufs=3)
                nc.scalar.activation(lse[:, :], ssum[:, :],
                                     mybir.ActivationFunctionType.Ln)
                yout = workp.tile([P, NC_], F32, tag="lsm", bufs=3)
                nc.vector.tensor_tensor(
                    yout[:, :].rearrange("p (g f) -> p g f", g=NG, f=C_DIM),
                    tsub[:, :].rearrange("p (g f) -> p g f", g=NG, f=C_DIM),
                    lse[:, :].unsqueeze(2).broadcast_to([P, NG, C_DIM]),
                    op=mybir.AluOpType.subtract,
                )
                nc.sync.dma_start(
                    out=y_d[:, :].rearrange("(p g) f -> p (g f)", p=P),
                    in_=yout[:, :],
                )

            for r in range(repeat):
                pipeline(gsh_all[r], gfull_all[r])

    nc.compile()
    return nc


_CACHE = {}

LAST_RESULT = None


def kernel(x, edge_index, W1, b1, W2, b2, Wl, bl):
    global LAST_RESULT
    x = np.asarray(x)
    edge_index = np.asarray(edge_index)
    n_nodes = x.shape[0]
    key = (n_nodes, edge_index.shape[1],
           bytes(np.asarray(edge_index[1, :64]).astype(np.int64)))
    if key not in _CACHE:
        plan = build_plan(edge_index, n_nodes)
        nc = build_bass(plan)
        _CACHE[key] = (plan, nc)
    plan, nc = _CACHE[key]

    in_maps = build_inputs(plan, x, np.asarray(W1), np.asarray(b1),
                           np.asarray(W2), np.asarray(b2),
                           np.asarray(Wl), np.asarray(bl))
    res = run_bass_kernel_spmd(nc, in_maps, core_ids=list(range(N_CORES)),
                               trace=False)
    LAST_RESULT = res
    y = np.empty((n_nodes, C_DIM), dtype=np.float32)
    for c in range(N_CORES):
        nl = plan.nodes_per_core[c]
        ranks = plan.usable[: len(nl)]
        yc = res.results[c]["y"]   # row index = pos = (r%128)*NG + r//128
        posn = (ranks % P) * NG + ranks // P
        y[nl] = yc[posn]
    return y
